# revision 34
# baseline (speedup 1.0000x reference)
"""DoubleAttention TRN2 Bass kernel.

Full inputs in, full outputs out. Data-parallel over batch: B=32 split as
4 batches per core across 8 NeuronCores; weights replicated.

Reference math per batch (C = Cout = dn = 512, N = H*W = 1024):
  A   = wA @ x + bA            [C, N]
  smB = softmax(wB @ x, n)     (bB drops: softmax shift-invariant)
  smV = softmax(wV @ x, n)     (bV drops)
  G   = A @ smB^T              [C, C]
  Z   = wR @ (G @ smV) + bR    [C, N]

Restructured: softmax rows sum to 1, so G = wA (x smB^T) + bA 1^T and
  Z = WRA (x EB^T) diag(rsB rsV) EV + (wR bA) (rsV^T EV) + bR 1^T
with WRA = wR wA (host-precomputed), rsB = 1/rowsum(EB), rsV likewise.
This removes the A-projection entirely and shrinks the [C,C]x[C,C]
product to half a projection: 144 512-col matmuls/batch vs 192.

Kernel phases (everything float32r on the PE):
  V: EV[d,n]   = exp(wV x)      natural layout + row expsums    (32 mm)
  B: EBT[n,d]  = exp(x^T wB^T)  x chunks stationary             (32 mm)
  M: M[c,d]    = (sum_n xT[n,c] EBT[n,d]) / sB[d]               (32 mm)
     (xT DMA'd from host; the 1/sB fold rides the evac as a
      broadcast multiply — every row of the ones-matmul PSUM
      tile already holds sB, so its reciprocal is a valid
      [128,512] operand and no partition transpose is needed)
  P: PT[d,o]   = sum_c M[c,d] WRAT[c,o]; evac folds the 1/sV
     scale and the +c[o]/sV[d] rank-1 term                      (16 mm)
  F: out[o,n]  = sum_d PT[d,o] EV[d,n] + bR[o]                  (32 mm)
  sB[d] via skewed DVE partial-sum tree + one ones-matmul.

x/xT and the projection weights stream in as bf16 (matmul speed is the
same 1 cyc/row, but the DMA head halves — batch 0's V phase was DMA
starved in fp32 and the resulting PE gaps held the clock at the mid
p-state). The P/F chain stays float32r end-to-end; PSUM is fp32.
"""

import numpy as np

B, C, N = 32, 512, 1024  # batch, channels, spatial (32*32)
H = W = 32
NCORES = 8
BPC = B // NCORES   # batches per core
KT = C // 128       # 4 contraction tiles
NT = N // 128       # 8 n-partition tiles
NS = N // 512       # 2 n free-dim spans

_CACHE = {}


def _build_nc():
    import concourse.bacc as bacc
    import concourse.mybir as mybir
    import concourse.tile as tile
    from concourse.alu_op_type import AluOpType

    F32 = mybir.dt.float32
    F32R = mybir.dt.float32r
    BF16 = mybir.dt.bfloat16
    AF = mybir.ActivationFunctionType

    nc = bacc.Bacc("TRN2", target_bir_lowering=False, debug=False,
                   num_devices=NCORES)
    x_d = nc.dram_tensor("x", [BPC, C, N], BF16, kind="ExternalInput").ap()
    xt_d = nc.dram_tensor("xt", [BPC, N, C], BF16, kind="ExternalInput").ap()
    wbt_d = nc.dram_tensor("wbt", [C, C], BF16, kind="ExternalInput").ap()
    wvt_d = nc.dram_tensor("wvt", [C, C], BF16, kind="ExternalInput").ap()
    wrat_d = nc.dram_tensor("wrat", [C, C], BF16, kind="ExternalInput").ap()
    cb_d = nc.dram_tensor("cb", [128, C], F32, kind="ExternalInput").ap()
    br_d = nc.dram_tensor("br", [128, KT], F32, kind="ExternalInput").ap()
    ones_d = nc.dram_tensor("ones", [128, 128], F32R, kind="ExternalInput").ap()
    o_d = nc.dram_tensor("o", [BPC, C, N], BF16, kind="ExternalOutput").ap()

    with tile.TileContext(nc) as tc:
        with tc.tile_pool(name="wp", bufs=1) as wp, \
             tc.tile_pool(name="xp", bufs=2) as xp, \
             tc.tile_pool(name="ip", bufs=1) as ip, \
             tc.tile_pool(name="op", bufs=1) as op_, \
             tc.tile_pool(name="sp", bufs=2) as sp, \
             tc.tile_pool(name="pp", bufs=8, space="PSUM") as pp:

            wbt = wp.tile([128, KT, C], BF16, tag="wbt")
            wvt = wp.tile([128, KT, C], BF16, tag="wvt")
            wrat = wp.tile([128, KT, C], BF16, tag="wrat")
            xs0 = xp.tile([128, KT, N], BF16, tag="xs")
            xt0 = xp.tile([128, NT, C], BF16, tag="xt")
            ones = wp.tile([128, 128], F32R, tag="ones")
            # Warm the PE clock gate while the head DMAs stream: each
            # fp32 matmul (4 cyc/row) runs ~1us at the startup clock, so
            # three of them bridge the ~6us from sequencer-preamble end
            # to batch 0's first data without delaying it.
            garb = wp.tile([128, 512], F32, tag="garb")
            nc.gpsimd.memset(garb[:], 1.0)
            psw = pp.tile([128, 512], F32, tag="mm")
            for _ in range(2):
                nc.tensor.matmul(psw[:], garb[:, 0:128], garb[:],
                                 start=True, stop=True)
            # All input loads trigger from the GpSimd sequencer: its DGE
            # dispatch is ~25ns/trigger vs ~650ns on SP, so every head
            # transfer is queued on the DMA engines by ~6us (the SP
            # serial chain used to stretch past 12us). SP keeps only the
            # output stores. Priority: B consumes xs-as-stationary in
            # n-quarter-slices against wbt; V needs wvt by ~10us, M
            # needs xt by ~17us, P needs wrat by ~24us.
            nc.gpsimd.dma_start(xs0[:, :, 0:256],
                                x_d[0, :, 0:256].rearrange(
                                    "(k p) n -> p k n", p=128))
            nc.gpsimd.dma_start(wbt[:], wbt_d.rearrange("(k p) c -> p k c",
                                                        p=128))
            for q in range(1, 4):
                qsl = slice(q * 256, (q + 1) * 256)
                nc.gpsimd.dma_start(xs0[:, :, qsl],
                                    x_d[0, :, qsl].rearrange(
                                        "(k p) n -> p k n", p=128))
            nc.gpsimd.dma_start(wvt[:], wvt_d.rearrange("(k p) c -> p k c",
                                                        p=128))
            cb = wp.tile([128, C], F32, tag="cb")
            br = wp.tile([128, KT], F32, tag="br")
            nc.gpsimd.dma_start(ones[:], ones_d[:])
            nc.gpsimd.dma_start(cb[:], cb_d[:])
            nc.gpsimd.dma_start(br[:], br_d[:])
            for h in range(NS):
                nc.gpsimd.dma_start(
                    xt0[:, h * KT:(h + 1) * KT, :],
                    xt_d[0, h * 512:(h + 1) * 512, :].rearrange(
                        "(t p) c -> p t c", p=128))
            nc.gpsimd.dma_start(wrat[:], wrat_d.rearrange("(k p) c -> p k c",
                                                          p=128))

            for b in range(BPC):
                if b == 0:
                    xs, xt = xs0, xt0
                else:
                    xs = xp.tile([128, KT, N], BF16, tag="xs")
                    xt = xp.tile([128, NT, C], BF16, tag="xt")
                    for h in range(NS):
                        hsl = slice(h * 512, (h + 1) * 512)
                        nc.gpsimd.dma_start(
                            xs[:, :, hsl],
                            x_d[b, :, hsl].rearrange("(k p) n -> p k n",
                                                     p=128))
                        nc.gpsimd.dma_start(
                            xt[:, h * KT:(h + 1) * KT, :],
                            xt_d[b, hsl, :].rearrange("(t p) c -> p t c",
                                                      p=128))

                ebt = ip.tile([128, NT, C], BF16, tag="ebt")
                ev = ip.tile([128, KT, N], BF16, tag="ev")
                m_ = ip.tile([128, KT, C], BF16, tag="m")
                pt_ = ip.tile([128, KT, C], BF16, tag="pt")
                av = sp.tile([128, KT, NS], F32, tag="av")
                svc = sp.tile([128, KT], F32, tag="svc")
                rsv = sp.tile([128, KT], F32, tag="rsv")
                rbb = sp.tile([128, C], F32, tag="rbb")
                tb = sp.tile([128, KT, C], F32, tag="tb")
                ebp = [sp.tile([128, C], F32R, tag=f"ebp{i}",
                               name=f"ebp{i}", bufs=1) for i in range(7)]
                os_ = op_.tile([128, KT, N], BF16, tag="os")

                # Phase B: EBT[n,d] per n-tile; skewed sB partial-sum
                # tree on DVE. B runs first: its exps + tree then have
                # the whole V phase of slack before the pss matmul needs
                # the total, and batch 0's first group only needs wbt
                # plus a quarter-slice of x.
                with nc.named_scope(f"B{b}"), nc.allow_low_precision(
                        reason="fp32r partials match the fp32r pipeline"):
                    for nt in range(NT):
                        nsl = slice(nt * 128, (nt + 1) * 128)
                        psb = pp.tile([128, C], F32, tag="mm")
                        for k in range(KT):
                            nc.tensor.matmul(psb[:], xs[:, k, nsl],
                                             wbt[:, k, :],
                                             start=(k == 0),
                                             stop=(k == KT - 1))
                        nc.scalar.activation(ebt[:, nt, :], psb[:], AF.Exp)
                        if nt == 1:
                            nc.vector.tensor_add(ebp[0][:], ebt[:, 0, :],
                                                 ebt[:, 1, :])
                        elif nt == 3:
                            nc.vector.tensor_add(ebp[1][:], ebt[:, 2, :],
                                                 ebt[:, 3, :])
                            nc.vector.tensor_add(ebp[2][:], ebp[0][:],
                                                 ebp[1][:])
                        elif nt == 5:
                            nc.vector.tensor_add(ebp[3][:], ebt[:, 4, :],
                                                 ebt[:, 5, :])
                        elif nt == 6:
                            nc.vector.tensor_add(ebp[4][:], ebp[3][:],
                                                 ebt[:, 6, :])
                            nc.vector.tensor_add(ebp[5][:], ebp[2][:],
                                                 ebp[4][:])
                        elif nt == 7:
                            nc.vector.tensor_add(ebp[6][:], ebp[5][:],
                                                 ebt[:, 7, :])

                # Phase V: EV[d,n] natural + per-row expsums. The sB
                # ones-matmul (pss[j,d] = sB[d] in every row j) slots in
                # mid-V: the B tree total is long done, so no PE stall,
                # and its reciprocal (a [128,512] broadcast operand) is
                # ready before the first M evac needs it.
                with nc.named_scope(f"V{b}"):
                    for h in range(NS):
                        hsl = slice(h * 512, (h + 1) * 512)
                        for dt in range(KT):
                            dsl = slice(dt * 128, (dt + 1) * 128)
                            psv = pp.tile([128, 512], F32, tag="mm")
                            for k in range(KT):
                                nc.tensor.matmul(psv[:], wvt[:, k, dsl],
                                                 xs[:, k, hsl],
                                                 start=(k == 0),
                                                 stop=(k == KT - 1))
                            nc.scalar.activation(ev[:, dt, hsl], psv[:],
                                                 AF.Exp,
                                                 accum_out=av[:, dt, h:h + 1])
                            if h == 0 and dt == KT - 1:
                                pss = pp.tile([128, 512], F32, tag="mm")
                                nc.tensor.matmul(pss[:], ones[:], ebp[6][:],
                                                 start=True, stop=True)
                                nc.vector.reciprocal_approx_fast(rbb[:],
                                                                 pss[:])
                    nc.vector.tensor_add(svc[:], av[:, :, 0], av[:, :, 1])
                    nc.vector.reciprocal(rsv[:], svc[:])
                    # tb[p,o] = c[o] * rsV[dt-chunk p] — the rank-1 term of
                    # the P evac; hoisted here (only needs rsv, not psp).
                    for dt in range(KT):
                        nc.vector.tensor_scalar_mul(tb[:, dt, :], cb[:],
                                                    rsv[:, dt:dt + 1])

                # Phase M: M[c,d] = (sum_n xT[n,c] EBT[n,d]) / sB[d].
                # The ones-matmul (pss[j,d] = sB[d] in every row j) goes
                # after the ct=0 group so the in-order PE stream doesn't
                # stall on the DVE tree total; its reciprocal is a
                # [128,512] broadcast operand the m evacs multiply by —
                # folding the d-scale here avoids any partition
                # transpose of sB.
                with nc.named_scope(f"M{b}"):
                    for ct in range(KT):
                        csl = slice(ct * 128, (ct + 1) * 128)
                        psm = pp.tile([128, C], F32, tag="mm")
                        for nt in range(NT):
                            nc.tensor.matmul(psm[:], xt[:, nt, csl],
                                             ebt[:, nt, :],
                                             start=(nt == 0),
                                             stop=(nt == NT - 1))
                        nc.vector.tensor_mul(m_[:, ct, :], psm[:], rbb[:])

                # Phase P: PT[d,o] = (psp + c)/sV; single fused evac op
                # (tb = cb*rsV was hoisted to the end of phase V).
                with nc.named_scope(f"P{b}"):
                    for dt in range(KT):
                        dsl = slice(dt * 128, (dt + 1) * 128)
                        psp = pp.tile([128, C], F32, tag="mm")
                        for ct in range(KT):
                            nc.tensor.matmul(psp[:], m_[:, ct, dsl],
                                             wrat[:, ct, :],
                                             start=(ct == 0),
                                             stop=(ct == KT - 1))
                        nc.vector.scalar_tensor_tensor(
                            pt_[:, dt, :], psp[:], rsv[:, dt:dt + 1],
                            tb[:, dt, :], op0=AluOpType.mult,
                            op1=AluOpType.add)

                # Phase F: out[o,n] = PT^T EV + bR (bias via ACT), DMA out
                with nc.named_scope(f"F{b}"):
                    for ot in range(KT):
                        osl = slice(ot * 128, (ot + 1) * 128)
                        for h in range(NS):
                            hsl = slice(h * 512, (h + 1) * 512)
                            psf = pp.tile([128, 512], F32, tag="mm")
                            for dt in range(KT):
                                nc.tensor.matmul(psf[:], pt_[:, dt, osl],
                                                 ev[:, dt, hsl],
                                                 start=(dt == 0),
                                                 stop=(dt == KT - 1))
                            nc.vector.tensor_scalar_add(os_[:, ot, hsl],
                                                        psf[:],
                                                        br[:, ot:ot + 1])
                            nc.sync.dma_start(
                                o_d[b, ot * 128:(ot + 1) * 128,
                                    h * 512:(h + 1) * 512],
                                os_[:, ot, hsl])
    nc.compile()
    return nc


def _in_maps(x, wA, bA, wB, wV, wR, bR):
    import ml_dtypes
    bf16 = ml_dtypes.bfloat16
    xr = x.reshape(B, C, N).astype(bf16)
    xtr = np.ascontiguousarray(xr.transpose(0, 2, 1))
    wbt = np.ascontiguousarray(wB.T.astype(bf16))
    wvt = np.ascontiguousarray(wV.T.astype(bf16))
    wrat = np.ascontiguousarray((wR @ wA).T.astype(bf16))
    cvec = (wR @ bA).astype(np.float32)
    cb = np.ascontiguousarray(
        np.broadcast_to(cvec.reshape(1, C), (128, C)), dtype=np.float32)
    br = np.ascontiguousarray(bR.reshape(KT, 128).T, dtype=np.float32)
    ones = np.ones((128, 128), dtype=np.float32)
    maps = []
    for i in range(NCORES):
        maps.append({
            "x": np.ascontiguousarray(xr[i * BPC:(i + 1) * BPC]),
            "xt": np.ascontiguousarray(xtr[i * BPC:(i + 1) * BPC]),
            "wbt": wbt, "wvt": wvt, "wrat": wrat,
            "cb": cb, "br": br, "ones": ones,
        })
    return maps


def kernel(x, wA, bA, wB, bB, wV, bV, wR, bR):
    from concourse.bass_utils import run_bass_kernel_spmd
    if "nc" not in _CACHE:
        _CACHE["nc"] = _build_nc()
    nc = _CACHE["nc"]
    maps = _in_maps(x, wA, bA, wB, wV, wR, bR)
    res = run_bass_kernel_spmd(nc, maps, list(range(NCORES)))
    out = np.concatenate([res.results[i]["o"] for i in range(NCORES)], axis=0)
    return out.reshape(B, C, H, W).astype(np.float32)
